# revision 45
# baseline (speedup 1.0000x reference)
"""Trainium2 Bass kernel for nn_DeformationCorrector.

Math (per particle, F = [[a,b],[c,d]], det F > 0 for this data):
  closed-form 2x2 SVD:  y1 = (a+d)^2 + (c-b)^2,  y2 = (a-d)^2 + (c+b)^2
    sq1 = sqrt(y1), sq2 = sqrt(y2);  sigma1 = (sq1+sq2)/2, sigma2 = (sq1-sq2)/2
  polar rotation R = U@Vh = [[p,-q],[q,p]],  p = (a+d)/sq1, q = (c-b)/sq1
  features (dedup; -1 shifts folded into b1):
    [sq1+sq2, sq1-sq2, a^2+c^2, ab+cd, b^2+d^2, ad-bc]  @ W1eff + b1eff
  MLP 6->128->128->3 (symmetrized W3), then delta = R @ x_sym, out = delta + F.

Distribution: pure data parallel over 8 cores, contiguous shards, weights
replicated. Layout conversions (particle-major elementwise <-> feature-major
matmul) go through DRAM round trips.

Schedule notes:
  - PE HAM clock gate: the array idles at 1.2 GHz until it sees ~3.4us of
    sustained busy, and re-throttles after a similar idle window. A chain of
    back-to-back dummy matmuls (gated on an early featd store) warms it right
    before stage-2; the stage-2 stream then keeps gaps short enough to stay
    at 2.4 GHz.
  - stage-2 unit u = 2048 particles (4 groups x T):
      PE:  [L1(u) 4-row-pack -> z1 (4 banks)] [L2(u-1) x4 -> z2a|z2b (2+2
      banks)] [L3(u-2) 4-col-pack -> z2a[:, :T] after relu2a freed it]
      ACT: relu1(u) as ONE [128,2048] instr; most drains.
      DVE: relu2a/b(u-1) as two [128,1024] instrs; some drains.
    PSUM: z1 4 banks + z2a 2 + z2b 2 = 8 exactly; x parks in z2a bank 0
    between relu2a(u-1) and L2(u)'s g0 write (WAR-ordered by Tile).
  - stage-1 of block 0 (head): h0 on DVE, h1 on GpSimd, finishing ACT/DVE
    bits inline; block >0: bulk on GpSimd, but its sqrt/clamp/recip +
    feature finish + featd stores are EMITTED mid-stage-2 (K_DEFER) so they
    don't head-of-line-block the ACT/DVE queues during pipeline fill.
  - stage-3 of block<last on GpSimd (overlaps stage-2), last block split
    DVE/GpSimd after the last unit.
"""

import os
from contextlib import ExitStack

import numpy as np

import concourse.bass as bass
import concourse.bacc as bacc
import concourse.tile as tile
from concourse.tile_rust import add_dep_helper
from concourse import mybir
from concourse.bass_utils import run_bass_kernel_spmd

NCORES = 8
P = 128
T = 512                 # matmul moving free dim (one PSUM bank of fp32)
CB = 512                # particles per partition per block
BLK = P * CB            # 65536 particles per block
NBLK = 2
NPC = NBLK * BLK        # 131072 particles per core (padded)
NTOT = NCORES * NPC     # 1048576
N = 1_000_000
HID = 128

CHUNKS_PER_BLK = BLK // T      # 128
GROUPS_PER_BLK = CHUNKS_PER_BLK // 4   # 32 (4 chunks per group: row/col packing)

FP32 = mybir.dt.float32
F32R = mybir.dt.float32r
BF16 = mybir.dt.bfloat16
AF = mybir.ActivationFunctionType
OP = mybir.AluOpType

# knobs
K_WARM_MM = int(os.environ.get("K_WARM_MM", "12"))        # PE warmup matmuls
K_WARM2 = int(os.environ.get("K_WARM2", "6"))             # fill-gap insurance MMs
K_FILL = int(os.environ.get("K_FILL", "0"))               # steady-state HAM filler MMs
K_DRAIN_ACT_OF8 = int(os.environ.get("K_DRAIN_ACT_OF8", "7"))  # drain split
K_DEFER = int(os.environ.get("K_DEFER", "12"))            # unit idx for b>0 s1 finish
K_S3B0 = int(os.environ.get("K_S3B0", "40"))              # unit idx for early-block s3

_built = {}
_last_results = None


def build_program(nblk=NBLK, cb=CB, dbg=False):
    global NBLK, CB
    NBLK_s, CB_s = NBLK, CB
    NBLK, CB = nblk, cb
    BLK_l = P * cb
    NPC_l = nblk * BLK_l
    try:
        nc = _build_impl(nblk, cb, BLK_l, NPC_l, dbg)
    finally:
        NBLK, CB = NBLK_s, CB_s
    return nc


def _build_impl(NBLK, CB, BLK, NPC, dbg=False):
    assert CB == T, 'g-major DRAM layout requires CB == T'
    CHUNKS_PER_BLK = BLK // T
    GROUPS_PER_BLK = CHUNKS_PER_BLK // 4
    SB = 4                              # units per superblock (featfm/x DMA batch)
    n_super = GROUPS_PER_BLK // SB      # 8
    nc = bacc.Bacc(trn_type="TRN2")

    F_in = nc.dram_tensor("F", [NPC, 4], FP32, kind="ExternalInput")
    W1S_in = nc.dram_tensor("W1S", [P, P], BF16, kind="ExternalInput")
    W2_in = nc.dram_tensor("W2", [P, P], BF16, kind="ExternalInput")
    W3S_in = nc.dram_tensor("W3S", [P, 32], BF16, kind="ExternalInput")
    B1_in = nc.dram_tensor("B1", [P, 1], FP32, kind="ExternalInput")
    B2_in = nc.dram_tensor("B2", [P, 1], FP32, kind="ExternalInput")
    B3S_in = nc.dram_tensor("B3S", [P, 1], FP32, kind="ExternalInput")
    OUT = nc.dram_tensor("OUT", [NPC, 4], FP32, kind="ExternalOutput")

    with tile.TileContext(nc) as tc, ExitStack() as ctx:
        consts = ctx.enter_context(tc.tile_pool(name="consts", bufs=1))
        fblk = ctx.enter_context(tc.tile_pool(name="fblk", bufs=NBLK))
        scr = ctx.enter_context(tc.tile_pool(name="scr", bufs=1))
        featp = ctx.enter_context(tc.tile_pool(name="featp", bufs=NBLK))
        dramp = ctx.enter_context(tc.tile_pool(name="dramp", bufs=NBLK, space="DRAM"))
        fmp = ctx.enter_context(tc.tile_pool(name="fmp", bufs=2))
        hp = ctx.enter_context(tc.tile_pool(name="hp", bufs=2))
        xp = ctx.enter_context(tc.tile_pool(name="xp", bufs=2))
        outp = ctx.enter_context(tc.tile_pool(name="outp", bufs=2))
        psz1 = ctx.enter_context(tc.tile_pool(name="psz1", bufs=1, space="PSUM"))
        psz2 = ctx.enter_context(tc.tile_pool(name="psz2", bufs=1, space="PSUM"))

        # ---- constants ----
        w1s_sb = consts.tile([P, P], BF16)
        nc.sync.dma_start(out=w1s_sb[:], in_=W1S_in[:, :])
        w2_sb = consts.tile([P, P], BF16)
        nc.sync.dma_start(out=w2_sb[:], in_=W2_in[:, :])
        w3s_sb = consts.tile([P, 32], BF16)
        nc.sync.dma_start(out=w3s_sb[:], in_=W3S_in[:, :])
        b1_sb = consts.tile([P, 1], FP32)
        nc.sync.dma_start(out=b1_sb[:], in_=B1_in[:, :])
        b2_sb = consts.tile([P, 1], FP32)
        nc.sync.dma_start(out=b2_sb[:], in_=B2_in[:, :])
        b3s_sb = consts.tile([P, 1], FP32)
        nc.sync.dma_start(out=b3s_sb[:], in_=B3S_in[:, :])
        warm_sb = consts.tile([P, T], BF16)
        nc.vector.memset(warm_sb[:], 0.5)

        f_tiles = []
        p_tiles = []
        q_tiles = []
        featd_tiles = []
        xd_tiles = []
        s1_state = {}
        warm_trigger = [None]  # featd store inst that gates the PE warmup chain

        last_q = {"pe": None, "act": None, "dve": None, "gps": None}

        def chain(qn, inst):
            # pin an engine FIFO to an explicit order: same-engine edges cost
            # nothing at runtime but stop the scheduler from reordering the
            # queue based on its (imperfect) readiness model
            if inst is None:
                return
            if last_q[qn] is not None:
                add_dep_helper(inst.ins, last_q[qn].ins, reason=f"{qn} order chain")
            last_q[qn] = inst

        H = CB // 2

        # ============ stage 1 setup: tiles + F loads ============
        def emit_s1_setup(b):
            f_sb = fblk.tile([P, 4 * CB], FP32, tag="F", name=f"f_sb{b}")
            F_bv = F_in[:, :].rearrange("(b i g j) k -> b i g (j k)", b=NBLK, i=32, g=4)[b]
            for hh in range(2):
                for g in range(4):
                    nc.sync.dma_start(
                        out=f_sb[32 * g : 32 * g + 32, hh * 2 * CB : (hh + 1) * 2 * CB],
                        in_=F_bv[:, g, hh * 2 * CB : (hh + 1) * 2 * CB],
                    )
            f_tiles.append(f_sb)
            fr = f_sb.rearrange("p (c k) -> p c k", k=4)
            fr2 = f_sb.rearrange("p (c k2 k) -> p c k2 k", k2=2, k=2)

            feat_sb = featp.tile([P, 6 * CB], BF16, tag="feat", name=f"feat_sb{b}")
            fv = feat_sb.rearrange("p (f c) -> p f c", f=6)
            sq_sb = scr.tile([P, 4 * CB], FP32, tag="sq", name=f"sq_sb{b}")
            sqr = sq_sb.rearrange("p (c k) -> p c k", k=4)
            pp_sb = scr.tile([P, 2 * CB], FP32, tag="pp", name=f"pp_sb{b}")
            ppv = pp_sb.rearrange("p (c k2) -> p c k2", k2=2)
            ad_sb = scr.tile([P, CB], FP32, tag="ad", name=f"ad_sb{b}")
            bc_sb = scr.tile([P, CB], FP32, tag="bc", name=f"bc_sb{b}")
            f5f_sb = scr.tile([P, CB], FP32, tag="f5f", name=f"f5f_sb{b}")
            m_sb = scr.tile([P, CB], FP32, tag="m", name=f"m_sb{b}")
            # y1/y2/s/v are written in partA (emitted for every block up
            # front) but read in partB (block >0 deferred into stage-2), so
            # they need per-block buffers to survive the pool rotation.
            y1_sb = scr.tile([P, CB], FP32, tag="y1", name=f"y1_sb{b}", bufs=NBLK)
            y2_sb = scr.tile([P, CB], FP32, tag="y2", name=f"y2_sb{b}", bufs=NBLK)
            sq1_sb = scr.tile([P, CB], FP32, tag="sq1", name=f"sq1_sb{b}")
            sq2_sb = scr.tile([P, CB], FP32, tag="sq2", name=f"sq2_sb{b}")
            s_sb = scr.tile([P, CB], FP32, tag="s", name=f"s_sb{b}", bufs=NBLK)
            v_sb = scr.tile([P, CB], FP32, tag="v", name=f"v_sb{b}", bufs=NBLK)
            rinv_sb = scr.tile([P, CB], FP32, tag="rinv", name=f"rinv_sb{b}")
            p_sb = fblk.tile([P, CB], FP32, tag="p", name=f"p_sb{b}")
            q_sb = fblk.tile([P, CB], FP32, tag="q", name=f"q_sb{b}")
            p_tiles.append(p_sb)
            q_tiles.append(q_sb)

            featd = dramp.tile([24, BLK // 4], BF16, tag="featd", name=f"featd{b}")
            featd_tiles.append(featd)
            xd = dramp.tile([12, BLK // 4], FP32, tag="xd", name=f"xd{b}")
            xd_tiles.append(xd)

            st = {"f_sb": f_sb, "feat_sb": feat_sb, "fv": fv, "y1": y1_sb,
                  "y2": y2_sb, "sq1": sq1_sb, "sq2": sq2_sb, "s": s_sb,
                  "v": v_sb, "rinv": rinv_sb, "p": p_sb, "q": q_sb,
                  "featd": featd, "fr": fr, "fr2": fr2, "sqr": sqr,
                  "ppv": ppv, "ad": ad_sb, "bc": bc_sb, "f5f": f5f_sb,
                  "m": m_sb}
            s1_state[b] = st

        # ---- part A: quadratic feature math for one column half ----
        def emit_s1_partA(b, h):
            st = s1_state[b]
            fr, fr2, sqr, ppv = st["fr"], st["fr2"], st["sqr"], st["ppv"]
            ad_sb, bc_sb, f5f_sb, m_sb = st["ad"], st["bc"], st["f5f"], st["m"]
            y1_sb, y2_sb, s_sb, v_sb = st["y1"], st["y2"], st["s"], st["v"]
            fv = st["fv"]
            # block 0: h0 on DVE, h1 on GpSimd (parallel head halves);
            # later blocks fully GpSimd (overlap with stage-2)
            use_gps = (b > 0) or (h == 1)
            e = nc.gpsimd if use_gps else nc.vector
            qn = "gps" if use_gps else "dve"
            cs = slice(h * H, (h + 1) * H)
            av, bv_, cv, dv = (fr[:, cs, k] for k in range(4))
            ac = fr2[:, cs, :, 0]
            bd = fr2[:, cs, :, 1]
            aa, bb, cc, dd = (sqr[:, cs, k] for k in range(4))
            ops = []
            ops.append(e.tensor_tensor(out=sqr[:, cs, :], in0=fr[:, cs, :], in1=fr[:, cs, :], op=OP.mult))
            ops.append(e.tensor_tensor(out=ppv[:, cs, :], in0=ac, in1=bd, op=OP.mult))
            ops.append(e.tensor_tensor(out=fv[:, 3, cs], in0=ppv[:, cs, 0], in1=ppv[:, cs, 1], op=OP.add))
            ops.append(e.tensor_tensor(out=ad_sb[:, cs], in0=av, in1=dv, op=OP.mult))
            ops.append(e.tensor_tensor(out=bc_sb[:, cs], in0=bv_, in1=cv, op=OP.mult))
            ops.append(e.tensor_tensor(out=f5f_sb[:, cs], in0=ad_sb[:, cs], in1=bc_sb[:, cs], op=OP.subtract))
            ops.append(e.tensor_copy(out=fv[:, 5, cs], in_=f5f_sb[:, cs]))
            ops.append(e.tensor_tensor(out=fv[:, 2, cs], in0=aa, in1=cc, op=OP.add))
            ops.append(e.tensor_tensor(out=fv[:, 4, cs], in0=bb, in1=dd, op=OP.add))
            ops.append(e.tensor_tensor(out=m_sb[:, cs], in0=fv[:, 2, cs], in1=fv[:, 4, cs], op=OP.add))
            if use_gps:
                d2 = ad_sb  # dead after f5; reuse as 2*f5 scratch
                ops.append(e.tensor_tensor(out=d2[:, cs], in0=f5f_sb[:, cs], in1=f5f_sb[:, cs], op=OP.add))
                ops.append(e.tensor_tensor(out=y1_sb[:, cs], in0=m_sb[:, cs], in1=d2[:, cs], op=OP.add))
                ops.append(e.tensor_tensor(out=y2_sb[:, cs], in0=m_sb[:, cs], in1=d2[:, cs], op=OP.subtract))
            else:
                ops.append(e.scalar_tensor_tensor(
                    out=y1_sb[:, cs], in0=f5f_sb[:, cs], scalar=2.0, in1=m_sb[:, cs],
                    op0=OP.mult, op1=OP.add))
                ops.append(e.scalar_tensor_tensor(
                    out=y2_sb[:, cs], in0=f5f_sb[:, cs], scalar=-2.0, in1=m_sb[:, cs],
                    op0=OP.mult, op1=OP.add))
            ops.append(e.tensor_tensor(out=s_sb[:, cs], in0=av, in1=dv, op=OP.add))
            ops.append(e.tensor_tensor(out=v_sb[:, cs], in0=cv, in1=bv_, op=OP.subtract))
            for o in ops:
                chain(qn, o)

        # ===== stage 1 part B: sqrt/recip + feature finish + featd stores =====
        # For b>0 this is emitted mid-stage-2 so the ACT/DVE FIFO work lands
        # after the early relu1/relu2 instructions instead of blocking them.
        def emit_s1_partB(b, h):
            st = s1_state[b]
            fv = st["fv"]
            y1_sb, y2_sb = st["y1"], st["y2"]
            sq1_sb, sq2_sb = st["sq1"], st["sq2"]
            s_sb, v_sb, rinv_sb = st["s"], st["v"], st["rinv"]
            p_sb, q_sb = st["p"], st["q"]
            feat_sb, featd = st["feat_sb"], st["featd"]
            use_gps = (b > 0) or (h == 1)
            e = nc.gpsimd if use_gps else nc.vector
            qn = "gps" if use_gps else "dve"
            cs = slice(h * H, (h + 1) * H)
            cl = nc.vector.tensor_scalar(
                out=y2_sb[:, cs], in0=y2_sb[:, cs], scalar1=0.0, scalar2=None, op0=OP.max)
            chain("dve", cl)
            sq1i = nc.scalar.activation(out=sq1_sb[:, cs], in_=y1_sb[:, cs], func=AF.Sqrt)
            chain("act", sq1i)
            sq2i = nc.scalar.activation(out=sq2_sb[:, cs], in_=y2_sb[:, cs], func=AF.Sqrt)
            chain("act", sq2i)
            ri = nc.vector.reciprocal_approx_fast(out=rinv_sb[:, cs], in_=sq1_sb[:, cs])
            chain("dve", ri)
            ops = []
            ops.append(e.tensor_tensor(out=fv[:, 0, cs], in0=sq1_sb[:, cs], in1=sq2_sb[:, cs], op=OP.add))
            ops.append(e.tensor_tensor(out=fv[:, 1, cs], in0=sq1_sb[:, cs], in1=sq2_sb[:, cs], op=OP.subtract))
            ops.append(e.tensor_tensor(out=p_sb[:, cs], in0=s_sb[:, cs], in1=rinv_sb[:, cs], op=OP.mult))
            ops.append(e.tensor_tensor(out=q_sb[:, cs], in0=v_sb[:, cs], in1=rinv_sb[:, cs], op=OP.mult))
            for o in ops:
                chain(qn, o)
            # featd stores: always from the GpSimd queue -- SWDGE handles
            # these scatter patterns much faster than the HW-DGE path the
            # sync/scalar queues use (measured ~2.3us/store vs well under
            # 1us), and it keeps them off the sync F-load path.
            eq = nc.gpsimd
            qd = "gps"
            for g in range(4):
                stq = eq.dma_start(
                    out=featd[6 * g : 6 * g + 6, :].rearrange("f (i j) -> i f j", j=T)[:, :, cs],
                    in_=feat_sb[32 * g : 32 * g + 32, :].rearrange("i (f j) -> i f j", j=T)[:, :, cs],
                )
                chain(qd, stq)
            if b == 0 and h == 0 and warm_trigger[0] is None:
                warm_trigger[0] = stq

        for b in range(NBLK):
            emit_s1_setup(b)
        for h in range(2):
            emit_s1_partA(0, h)
            emit_s1_partB(0, h)
        for b in range(1, NBLK):
            for h in range(2):
                emit_s1_partA(b, h)

        # ============ stage 3: particle-major backend ============
        # Emitted per block: early blocks are emitted MID-stage-2 (K_S3B0) so
        # their xs loads don't queue behind every featfm load on the sync
        # FIFO (emission order = queue order) and their GpSimd compute truly
        # overlaps stage-2; only the last block runs in the tail.
        def emit_s3(b):
            # tail block splits column-spans across DVE || GpSimd; earlier
            # blocks run fully on gpsimd (overlapped with stage 2).
            offload = b < NBLK - 1

            xd = xd_tiles[b]
            f_sb = f_tiles[b]
            p_sb = p_tiles[b]
            q_sb = q_tiles[b]
            fr = f_sb.rearrange("p (c k) -> p c k", k=4)

            xs_all = xp.tile([P, 3 * CB], FP32, tag="xsall", name=f"xsall{b}")
            xs_v = xs_all.rearrange("p (k c) -> p k c", k=3)
            for g in range(4):
                nc.sync.dma_start(
                    out=xs_all[32 * g : 32 * g + 32, :].rearrange("i (k j) -> i k j", j=T),
                    in_=xd[3 * g : 3 * g + 3, :].rearrange("k (i j) -> i k j", j=T),
                )
            pall = scr.tile([P, 3 * CB], FP32, tag="pall", name=f"pall{b}")
            pall_v = pall.rearrange("p (k c) -> p k c", k=3)
            qall = scr.tile([P, 3 * CB], FP32, tag="qall", name=f"qall{b}")
            qall_v = qall.rearrange("p (k c) -> p k c", k=3)
            out_sb = outp.tile([P, 4 * CB], FP32, tag="out", name=f"out_sb{b}")
            ov = out_sb.rearrange("p (c k) -> p c k", k=4)
            t0 = scr.tile([P, CB], FP32, tag="t0", name=f"t0_{b}")
            t1 = scr.tile([P, CB], FP32, tag="t1", name=f"t1_{b}")
            t2 = scr.tile([P, CB], FP32, tag="t2", name=f"t2_{b}")
            t3 = scr.tile([P, CB], FP32, tag="t3", name=f"t3_{b}")
            # tail block: DVE is ~2x GpSimd, so give it the bigger share
            HS = CB if offload else (CB * 5) // 8
            spans = [(0, HS), (HS, CB)] if HS < CB else [(0, CB)]
            OUT_bv = OUT[:, :].rearrange("(b i g j) k -> b i g (j k)", b=NBLK, i=32, g=4)[b]
            for hi, (c0, c1) in enumerate(spans):
                e = nc.gpsimd if (offload or hi == 1) else nc.vector
                qn = "gps" if (offload or hi == 1) else "dve"
                cs = slice(c0, c1)
                W = c1 - c0
                ops = []
                ops.append(e.tensor_tensor(
                    out=pall_v[:, :, cs], in0=xs_v[:, :, cs],
                    in1=p_sb[:, cs].unsqueeze(1).to_broadcast([P, 3, W]), op=OP.mult,
                ))
                ops.append(e.tensor_tensor(
                    out=qall_v[:, :, cs], in0=xs_v[:, :, cs],
                    in1=q_sb[:, cs].unsqueeze(1).to_broadcast([P, 3, W]), op=OP.mult,
                ))
                ops.append(e.tensor_tensor(out=t0[:, cs], in0=pall_v[:, 0, cs], in1=qall_v[:, 1, cs], op=OP.subtract))
                ops.append(e.tensor_tensor(out=ov[:, cs, 0], in0=t0[:, cs], in1=fr[:, cs, 0], op=OP.add))
                ops.append(e.tensor_tensor(out=t1[:, cs], in0=pall_v[:, 1, cs], in1=qall_v[:, 2, cs], op=OP.subtract))
                ops.append(e.tensor_tensor(out=ov[:, cs, 1], in0=t1[:, cs], in1=fr[:, cs, 1], op=OP.add))
                ops.append(e.tensor_tensor(out=t2[:, cs], in0=qall_v[:, 0, cs], in1=pall_v[:, 1, cs], op=OP.add))
                ops.append(e.tensor_tensor(out=ov[:, cs, 2], in0=t2[:, cs], in1=fr[:, cs, 2], op=OP.add))
                ops.append(e.tensor_tensor(out=t3[:, cs], in0=qall_v[:, 1, cs], in1=pall_v[:, 2, cs], op=OP.add))
                ops.append(e.tensor_tensor(out=ov[:, cs, 3], in0=t3[:, cs], in1=fr[:, cs, 3], op=OP.add))
                for o in ops:
                    chain(qn, o)
            for g in range(4):
                nc.sync.dma_start(out=OUT_bv[:, g, :], in_=out_sb[32 * g : 32 * g + 32, :])

        # ============ PE warmup: back-to-back dummy matmuls ============
        warm_mms = []
        if K_WARM_MM > 0:
            warm_ps = psz1.tile([P, 4 * T], FP32, tag="z1", name="z1_warm")
            for i in range(K_WARM_MM):
                mm = nc.tensor.matmul(
                    out=warm_ps[:, :T], lhsT=w2_sb[:], rhs=warm_sb[:]
                )
                if i == 0 and warm_trigger[0] is not None:
                    add_dep_helper(mm.ins, warm_trigger[0].ins, reason="warmup gate")
                warm_mms.append(mm)

        # ============ stage 2: feature-major MLP (software-pipelined units) ============
        units = [
            (b, s, j)
            for b in range(NBLK)
            for s in range(n_super)
            for j in range(SB)
        ]
        featfm_tiles = {}   # (b, s) -> featfm tile
        xsb_tiles = {}      # (b, s) -> x superblock drain tile
        PREFETCH = 5

        def emit_featfm(k):
            b, s, j = units[k]
            if (b, s) in featfm_tiles:
                return
            featd = featd_tiles[b]
            featfm = fmp.tile([P, SB * T], BF16, tag="featfm", name=f"ffm{b}_{s}")
            for g in range(4):
                nc.sync.dma_start(
                    out=featfm[32 * g : 32 * g + 6, :],
                    in_=featd[6 * g : 6 * g + 6, SB * T * s : SB * T * (s + 1)],
                )
            featfm_tiles[(b, s)] = featfm

        ctx1 = None  # unit u-1: (b, s, j, h1)
        ctx2 = None  # unit u-2: (b, s, j, h2a, h2b)
        cu = 0     # unit counter
        cdr = 0    # drain counter

        if warm_mms:
            last_q["pe"] = warm_mms[-1]
            for a, bm in zip(warm_mms, warm_mms[1:]):
                add_dep_helper(bm.ins, a.ins, reason="warmup chain")

        for idx in range(len(units) + 2):
            # ---- deferred stage-1 finish for overlapped blocks ----
            if idx == K_DEFER:
                for b in range(1, NBLK):
                    for hh_ in range(2):
                        emit_s1_partB(b, hh_)
            if idx == K_S3B0:
                for b in range(NBLK - 1):
                    emit_s3(b)

            # ---- prefetch featfm a few units ahead ----
            for k in range(idx, min(idx + PREFETCH + 1, len(units))):
                emit_featfm(k)

            # ---- L1 + relu1 of current unit ----
            cur = None
            l1_mms = []
            r1 = None
            if idx < len(units):
                b, s, j = units[idx]
                featfm = featfm_tiles[(b, s)]
                ffm_gv = featfm.rearrange("(g r) c -> g r c", g=4)
                z1 = psz1.tile([P, 4 * T], FP32, tag="z1", name=f"z1_{b}_{s}_{j}")
                for g in range(4):
                    mm = nc.tensor.matmul(
                        out=z1[:, g * T : (g + 1) * T],
                        lhsT=w1s_sb[32 * g : 32 * g + 6, :],
                        rhs=ffm_gv[g, :6, j * T : (j + 1) * T],
                        tile_position=(32 * g, 0),
                    )
                    l1_mms.append(mm)
                if idx == 0 and K_WARM2 > 0:
                    # fill-gap insurance: the pipeline-fill wait (relu1(0)
                    # before L2(0) can start) is long enough for the HAM to
                    # re-throttle; keep the PE streaming through it. The
                    # dummy tile rotates ahead of the first real z2b use.
                    wz = psz2.tile([P, 2 * T], FP32, tag="z2b", name="z2warm0")
                    for _ in range(K_WARM2):
                        wmm = nc.tensor.matmul(
                            out=wz[:, :T], lhsT=w2_sb[:], rhs=warm_sb[:]
                        )
                        l1_mms.append(wmm)
                h1 = hp.tile([P, 4 * T], BF16, tag="h1", name=f"h1_{b}_{s}_{j}", bufs=2)
                r1 = nc.scalar.activation(
                    out=h1[:], in_=z1[:], func=AF.Relu, bias=b1_sb[:]
                )
                if j == 0:
                    xsb_tiles[(b, s)] = xp.tile(
                        [P, SB * T], FP32, tag="xsb", name=f"xsb{b}_{s}"
                    )
                cur = (b, s, j, h1)
                cu += 1

            # ---- L2 + relu2 of unit u-1 ----
            nxt2 = None
            l2_mms = []
            fill_mms = []
            r2a = r2b = None
            z2a = None
            if ctx1 is not None:
                pb, ps, pj, ph1 = ctx1
                z2a = psz2.tile([P, 2 * T], FP32, tag="z2a", name=f"z2a_{pb}_{ps}_{pj}")
                z2b = psz2.tile([P, 2 * T], FP32, tag="z2b", name=f"z2b_{pb}_{ps}_{pj}")
                for g in range(4):
                    zt = z2a if g < 2 else z2b
                    mm = nc.tensor.matmul(
                        out=zt[:, (g % 2) * T : (g % 2 + 1) * T], lhsT=w2_sb[:],
                        rhs=ph1[:, g * T : (g + 1) * T],
                    )
                    l2_mms.append(mm)
                h2a = hp.tile([P, 2 * T], BF16, tag="h2a", name=f"h2a_{pb}_{ps}_{pj}", bufs=2)
                h2b = hp.tile([P, 2 * T], BF16, tag="h2b", name=f"h2b_{pb}_{ps}_{pj}", bufs=2)
                r2a = nc.vector.tensor_scalar(
                    out=h2a[:], in0=z2a[:], scalar1=b2_sb[:],
                    scalar2=0.0, op0=OP.add, op1=OP.max,
                )
                r2b = nc.vector.tensor_scalar(
                    out=h2b[:], in0=z2b[:], scalar1=b2_sb[:],
                    scalar2=0.0, op0=OP.add, op1=OP.max,
                )
                nxt2 = (pb, ps, pj, h2a, h2b)


            # ---- L3 + drain of unit u-2 (x parks in z2a bank 0) ----
            l3_mms = []
            drain = None
            drain_on_dve = False
            if ctx2 is not None:
                qb, qs, qj, qh2a, qh2b = ctx2
                if z2a is None:
                    z2a = psz2.tile([P, 2 * T], FP32, tag="z2a", name=f"z2a_tail{idx}")
                x_ps = z2a[:, :T]
                for g in range(4):
                    rhs_h2 = qh2a if g < 2 else qh2b
                    l3 = nc.tensor.matmul(
                        out=x_ps[32 * g : 32 * g + 32, :],
                        lhsT=w3s_sb[:, :],
                        rhs=rhs_h2[:, (g % 2) * T : (g % 2 + 1) * T],
                        tile_position=(0, 32 * g),
                    )
                    l3_mms.append(l3)
                x_sb = xsb_tiles[(qb, qs)]
                drain_on_dve = (cdr % 8) >= K_DRAIN_ACT_OF8
                if drain_on_dve:
                    drain = nc.vector.tensor_scalar(
                        out=x_sb[:, qj * T : (qj + 1) * T], in0=x_ps[:],
                        scalar1=b3s_sb[:], scalar2=None, op0=OP.add,
                    )
                else:
                    drain = nc.scalar.activation(
                        out=x_sb[:, qj * T : (qj + 1) * T], in_=x_ps[:],
                        func=AF.Identity, bias=b3s_sb[:],
                    )
                cdr += 1
                if qj == SB - 1:
                    xd = xd_tiles[qb]
                    for g in range(4):
                        nc.sync.dma_start(
                            out=xd[3 * g : 3 * g + 3, SB * T * qs : SB * T * (qs + 1)],
                            in_=x_sb[32 * g : 32 * g + 3, :],
                        )

            # ---- HAM fillers: pad the PE toward full occupancy ----
            # The clock gate re-throttles whenever the PE idles a meaningful
            # fraction of its activity window, and warm-PE stage-2 is only
            # ~60-80% PE-busy; once re-throttled the cold PE becomes the
            # critical path (matmuls take 2x). A dummy matmul after L3 keeps
            # the array streaming through the wait-for-relu tail gap. It
            # writes z2a's second (dead) bank; the drain's coarse tile dep
            # makes it wait on the filler, so keep the filler count tiny.
            if ctx1 is not None and K_FILL > 0:
                for _ in range(K_FILL):
                    fmm = nc.tensor.matmul(
                        out=z2a[:, T : 2 * T], lhsT=w2_sb[:], rhs=warm_sb[:]
                    )
                    fill_mms.append(fmm)

            # ---- wire the engine FIFOs for this steady-state step ----
            for mm in l1_mms + l2_mms + l3_mms + fill_mms:
                chain("pe", mm)
            chain("act", r1)
            if drain is not None and not drain_on_dve:
                chain("act", drain)
            chain("dve", r2a)
            chain("dve", r2b)
            if drain is not None and drain_on_dve:
                chain("dve", drain)

            ctx2 = nxt2
            ctx1 = cur

        emit_s3(NBLK - 1)

    nc.finalize()
    return nc


def prep_weights(W1, b1, W2, b2, W3, b3):
    """Host-side weight transforms (tiny)."""
    W1 = np.asarray(W1, np.float32)
    b1 = np.asarray(b1, np.float32)
    W2 = np.asarray(W2, np.float32)
    b2 = np.asarray(b2, np.float32)
    W3 = np.asarray(W3, np.float32)
    b3 = np.asarray(b3, np.float32)
    # features: [sq1+sq2, sq1-sq2, f2, f3, f4, f5]
    W1eff = np.stack(
        [0.5 * W1[0], 0.5 * W1[1], W1[2], W1[3] + W1[4], W1[5], W1[6]], axis=0
    )  # [6, 128]
    b1eff = b1 - (W1[0] + W1[1] + W1[2] + W1[5] + W1[6])
    W1S = np.zeros((P, P), np.float32)
    for g in range(4):
        W1S[32 * g : 32 * g + 6, :] = W1eff
    # symmetrized third layer: x_sym = [x00, (x01+x10)/2, x11]
    W3S = np.zeros((P, 32), np.float32)
    W3S[:, 0] = W3[:, 0]
    W3S[:, 1] = 0.5 * (W3[:, 1] + W3[:, 2])
    W3S[:, 2] = W3[:, 3]
    b3S3 = np.array([b3[0], 0.5 * (b3[1] + b3[2]), b3[3]], np.float32)
    B3S = np.zeros((P, 1), np.float32)
    for j in range(4):
        B3S[32 * j : 32 * j + 3, 0] = b3S3
    import ml_dtypes
    return {
        "W1S": W1S.astype(ml_dtypes.bfloat16),
        "W2": W2.astype(ml_dtypes.bfloat16),
        "W3S": W3S.astype(ml_dtypes.bfloat16),
        "B1": b1eff.reshape(P, 1).astype(np.float32),
        "B2": b2.reshape(P, 1).astype(np.float32),
        "B3S": B3S,
    }


def kernel(F, W1, b1, W2, b2, W3, b3):
    global _last_results
    F = np.asarray(F, np.float32).reshape(-1, 4)
    n = F.shape[0]
    assert n == N, f"expected {N} particles, got {n}"

    if "nc" not in _built:
        _built["nc"] = build_program()
    nc = _built["nc"]

    wmaps = prep_weights(W1, b1, W2, b2, W3, b3)
    Fpad = np.empty((NTOT, 4), np.float32)
    Fpad[:n] = F
    Fpad[n:] = np.array([1.0, 0.1, 0.0, 1.0], np.float32)

    in_maps = []
    for i in range(NCORES):
        m = {"F": np.ascontiguousarray(Fpad[i * NPC : (i + 1) * NPC])}
        m.update(wmaps)
        in_maps.append(m)

    res = run_bass_kernel_spmd(nc, in_maps, core_ids=list(range(NCORES)))
    _last_results = res
    out = np.concatenate([r["OUT"] for r in res.results], axis=0)[:n]
    return out.reshape(n, 2, 2).astype(np.float32)
